# revision 38
# baseline (speedup 1.0000x reference)
"""Causal self-attention Trainium2 kernel (B=256, T=256, C=384, 8 heads x 48).

Strategy: pure data-parallel over batch across 8 NeuronCores (32 batches per
core, no collectives). All layouts are arranged on the host so the device
kernel never transposes anything:

  - x is sent transposed per batch: xT [nb, C, T].
  - QK projection computes q^T / k^T in "feature-major" layout [o', tokens]
    with heads padded to 64-row blocks, augmented with 2 extra contraction
    rows that carry the q/k bias cross terms, so scores come out exactly
    (up to a per-head constant, which softmax ignores).  K_contract = 50.
  - Scores are computed transposed, S^T[k, q], per head with 2-head row-tiled
    matmul concurrency (head dim 50 <= 64 rows).  The fully-masked quadrant
    (k >= 128, q < 128) is never computed: the kx=1 score matmul only covers
    q in [128, 256).
  - Softmax skips the max-subtraction (inputs are well-scaled gaussians),
    exp on ACT straight PSUM->SBUF, causal mask applied multiplicatively
    afterwards with a single 0/1 triangular tile (the same 128x128 triangle
    serves both the kx=0/q<128 and kx=1/q>=128 blocks), split DVE/GpSimd.
  - The PV matmul consumes V in token-major layout (computed directly by
    swapping stationary/moving operands - no transpose), augmented with a
    leading ones column per head so row 0 of each head block is the softmax
    denominator.  Col-tiled 2-head concurrency; the kx=1 contribution only
    accumulates into the q >= 128 half.
  - Normalization: reciprocal of the denominator rows via exp(-ln(den)) on
    the Scalar engine (both functions live in the natural_log_exp_and_others
    ACT table set, so there is no table thrashing; the DVE iterative-divide
    reciprocal at ~8 cyc/elem on 1 partition was the old kernel's single
    largest cost).  GpSimd partition-broadcast of the bf16 reciprocal rows,
    one fused DVE multiply PSUM->SBUF.
  - Output projection consumes the attention output directly in its
    [c', token] layout; V-bias is folded into the projection bias on host.
    Its matmuls are emitted one batch late so they fill the PE pipeline
    while the next batch's softmax-normalize tail runs.

Matmuls run in bf16 (fp32 PSUM accumulation).
"""

import os
import sys

import numpy as np

try:
    import ml_dtypes

    BF16_NP = ml_dtypes.bfloat16
except ImportError:  # pragma: no cover
    BF16_NP = None

for _p in ("/opt/trn_rl_repo",):
    if os.path.isdir(_p) and _p not in sys.path:
        sys.path.insert(0, _p)

from contextlib import ExitStack

import concourse.bass as bass
import concourse.bacc as bacc
import concourse.tile as tile
from concourse import mybir
from concourse.bass_utils import run_bass_kernel_spmd

P = 128
T = 256
C = 384
NH = 8
HD = 48
KA = 50  # augmented contraction rows per head (48 + cq/ck row + ones row)
HB = 64  # padded head block stride
DT = mybir.dt.float32
BF = mybir.dt.bfloat16
AF = mybir.ActivationFunctionType
N_CORES = 8
B_FULL = 256
NB = B_FULL // N_CORES  # batches per core

EPS_PAD = 1e-20  # value for padded V columns (keeps reciprocal finite)

MMDT = BF


class _Bacc(bacc.Bacc):
    """Bacc whose ACT-table-load pass only offers this kernel's activation
    functions (Exp/Ln/Copy/Identity) in the one table set that contains all
    of them.  The stock pass greedily picks exp_and_others for Exp and
    natural_log for Ln, reloading tables (~2.7us each) twice per batch."""

    _KEEP = None

    def insert_act_table_loads(self):
        import bass_rust as _bass_rust

        from concourse.hw_specs import get_activation_tables

        has_activation = any(
            isinstance(i, mybir.InstActivation)
            for b in self.main_func.blocks
            for i in b.instructions
        )
        if not has_activation:
            return
        keep = {AF.Exp, AF.Ln, AF.Copy, AF.Identity}
        tables = []
        for name, fns in get_activation_tables(self.m.arch).items():
            if name != "natural_log_exp_and_others":
                fns = set(fns) - keep
            tables.append((name, fns))
        _bass_rust.insert_act_table_loads(self, tables)


def build_nc(nb: int = NB, debug: bool = False, repeat: int = 1, timing: bool = False):
    nc = _Bacc(None)

    # xT packed [3, 128, nb, T]: per (ci, partition) the (batch, token) block
    # is contiguous, so each per-pair DMA moves 1 KiB-contiguous rows.
    xT = nc.declare_dram_parameter("xT", [3, P, nb, T], MMDT, isOutput=False)
    wqk_d = nc.declare_dram_parameter("wqk", [3, P, 1024], MMDT, isOutput=False)
    wv_d = nc.declare_dram_parameter("wv", [3, P, 384], MMDT, isOutput=False)
    wp_d = nc.declare_dram_parameter("wp", [4, P, 384], MMDT, isOutput=False)
    bq_d = nc.declare_dram_parameter("bq", [P, 1], DT, isOutput=False)
    bk_d = nc.declare_dram_parameter("bk", [P, 1], DT, isOutput=False)
    tri0_d = nc.declare_dram_parameter("tri0", [P, 512], BF, isOutput=False)
    bp_d = nc.declare_dram_parameter("bp", [P, 384], DT, isOutput=False)
    vinit_d = nc.declare_dram_parameter("vinit", [P, 1024], BF, isOutput=False)
    if timing:
        y_d = nc.dram_tensor("y_int", [nb, T, C], BF)
        ydum_d = nc.declare_dram_parameter("ydum", [P, 4], DT, isOutput=True)
    else:
        y_d = nc.declare_dram_parameter("y", [nb, T, C], BF, isOutput=True)

    with tile.TileContext(nc) as tc, ExitStack() as ctx:
        const = ctx.enter_context(tc.tile_pool(name="const", bufs=1))
        xtp = ctx.enter_context(tc.tile_pool(name="xt", bufs=9))
        qkp = ctx.enter_context(tc.tile_pool(name="qkt", bufs=2))
        ptp = ctx.enter_context(tc.tile_pool(name="pt", bufs=2))
        aop = ctx.enter_context(tc.tile_pool(name="ao", bufs=2))
        yp = ctx.enter_context(tc.tile_pool(name="y", bufs=4))
        # PSUM: psA 2x[128,512] (QK waves + V waves + proj, one shared ring)
        # + psS [128,1024] + psO 2x[128,1024] (double-buffered attention
        # output: the next batch's PV never waits for the softmax-normalize
        # tail) = 16 KiB = all 8 banks.
        psA = ctx.enter_context(
            tc.tile_pool(name="psA", bufs=2, space=bass.MemorySpace.PSUM)
        )
        psS = ctx.enter_context(
            tc.tile_pool(name="psS", bufs=1, space=bass.MemorySpace.PSUM)
        )
        psO = ctx.enter_context(
            tc.tile_pool(name="psO", bufs=2, space=bass.MemorySpace.PSUM)
        )

        # ---- load constants (split across both HWDGE rings so the first
        # QK waves aren't gated on one serialized queue) -------------------
        wqk_sb = []
        wv_sb = []
        wp_sb = []
        for ci in range(3):
            t = const.tile([P, 1024], MMDT, tag=f"wqk{ci}")
            (nc.sync if ci % 2 == 0 else nc.scalar).dma_start(t[:], wqk_d[ci])
            wqk_sb.append(t)
        for ci in range(3):
            t = const.tile([P, 384], MMDT, tag=f"wv{ci}")
            (nc.sync if ci % 2 == 0 else nc.scalar).dma_start(t[:], wv_d[ci])
            wv_sb.append(t)
        for cc in range(4):
            t = const.tile([P, 384], MMDT, tag=f"wp{cc}")
            (nc.sync if cc % 2 == 0 else nc.scalar).dma_start(t[:], wp_d[cc])
            wp_sb.append(t)
        bq_sb = const.tile([P, 1], DT, tag="bq")
        nc.sync.dma_start(bq_sb[:], bq_d[:])
        bk_sb = const.tile([P, 1], DT, tag="bk")
        nc.sync.dma_start(bk_sb[:], bk_d[:])
        tri0_sb = const.tile([P, 512], BF, tag="tri0")
        nc.sync.dma_start(tri0_sb[:], tri0_d[:])
        bp_sb = const.tile([P, 384], DT, tag="bp")
        nc.sync.dma_start(bp_sb[:], bp_d[:])
        # persistent V tiles: the ones column + eps pads are loaded once;
        # per-batch projection copies only ever touch cols 1..48 of each
        # 64-block, so the constant columns survive.
        v_tiles = []
        for bb in range(2):
            vt = const.tile([P, 1024], BF, tag=f"vtile{bb}")
            nc.sync.dma_start(vt[:], vinit_d[:])
            v_tiles.append(vt)

        # [p, j, s, v]: four identical 128-wide triangles, matching the four
        # masked score regions (j head-of-pair, s kx-block) in one DVE op
        tri_m = tri0_sb[:].rearrange("p (j s v) -> p j s v", j=2, s=2, v=128)

        # xt prefetch queue: 3 feature-chunk tiles per pair, issued two
        # pairs ahead of use so the loads never gate the QK matmuls.
        n_pairs = (nb // 2) * repeat
        xt_queue = []

        def load_xt(pair_it):
            b0 = 2 * (pair_it % (nb // 2))
            tiles = []
            for ci in range(3):
                t = xtp.tile([P, 2 * T], MMDT, tag="xt")
                nc.sync.dma_start(
                    t[:].rearrange("p (b t) -> p b t", b=2),
                    xT[ci, :, b0 : b0 + 2, :],
                )
                tiles.append(t)
            xt_queue.append(tiles)

        for pre in range(min(2, n_pairs)):
            load_xt(pre)

        # deferred output-projection emitter: proj(b) runs while b+1's
        # softmax-normalize tail is on the other engines.
        pending_proj = []

        def flush_proj():
            while pending_proj:
                ao_prev, b_prev = pending_proj.pop(0)
                with tc.high_priority():
                    for tcx in range(2):
                        psy = psA.tile([P, 512], DT, tag="ps")
                        for cc in range(4):
                            nc.tensor.matmul(
                                psy[:, 0:384],
                                ao_prev[
                                    :,
                                    256 * cc + 128 * tcx : 256 * cc + 128 * tcx + 128,
                                ],
                                wp_sb[cc][:],
                                start=(cc == 0),
                                stop=(cc == 3),
                            )
                        # proj bias is folded into wp row 0 of head-block 0
                        # (the normalized denominator row of ao is exactly
                        # 1.0).  The evacuation runs on ACT: on the DVE it
                        # queues behind the softmax tail and delays the psA
                        # ring for the next pair's QK waves.
                        ysb = yp.tile([P, 384], BF, tag="y")
                        nc.scalar.activation(ysb[:], psy[:, 0:384], AF.Copy)
                        nc.sync.dma_start(
                            y_d[b_prev, 128 * tcx : 128 * tcx + 128, :], ysb[:]
                        )

        # ---- per-batch-pair pipeline ---------------------------------------
        assert nb % 2 == 0
        for bp_it in range(n_pairs):
            b0 = 2 * (bp_it % (nb // 2))
            xt = xt_queue.pop(0)
            if bp_it + 2 < n_pairs:
                load_xt(bp_it + 2)

            # QK projection: 8 waves of [128, 512], q/k interleaved so the
            # first score matmuls' inputs (q-oc0, k-oc0) evacuate earliest.
            # Evacuations split across ACT and DVE to balance engine load.
            qt = qkp.tile([P, 2048], MMDT, tag="qt")
            kt = qkp.tile([P, 2048], MMDT, tag="kt")
            for wave, (w, oc) in enumerate(
                (w, oc) for oc in range(4) for w in (0, 1)
            ):
                dst = qt if w == 0 else kt
                bias = bq_sb if w == 0 else bk_sb
                ps = psA.tile([P, 512], DT, tag="ps")
                for ci in range(3):
                    nc.tensor.matmul(
                        ps[:],
                        wqk_sb[ci][
                            :, 512 * w + 128 * oc : 512 * w + 128 * oc + 128
                        ],
                        xt[ci][:],
                        start=(ci == 0),
                        stop=(ci == 2),
                    )
                nc.vector.tensor_scalar_add(
                    dst[:, 512 * oc : 512 * oc + 512],
                    ps[:],
                    bias[:, 0:1],
                )

            # V in token-major layout (per batch), ones column + eps pads
            v_sbs = v_tiles
            for tch in range(4):
                bb, tcx = tch // 2, tch % 2
                psv = psA.tile([P, 512], DT, tag="ps")
                for ci in range(3):
                    nc.tensor.matmul(
                        psv[:, 0:384],
                        xt[ci][:, 256 * bb + 128 * tcx : 256 * bb + 128 * tcx + 128],
                        wv_sb[ci][:],
                        start=(ci == 0),
                        stop=(ci == 2),
                    )
                half = v_sbs[bb][:, 512 * tcx : 512 * tcx + 512].rearrange(
                    "p (h c) -> p h c", c=HB
                )
                psv_r = psv[:, 0:384].rearrange("p (h c) -> p h c", c=48)
                nc.vector.tensor_copy(half[:, :, 1:49], psv_r[:])

            for bb in range(2):
                b = b0 + bb
                v_sb = v_sbs[bb]
                # S^T per head pair + exp + mask + PV, interleaved per group.
                # pt per-group layout (within each j 512-block):
                #   [0:256)   kx=0 scores, all q
                #   [256:384) kx=1 scores, q in [128,256)
                #   [384:512) unused
                pt = ptp.tile([P, 4096], BF, tag="pt")
                pso = psO.tile([P, 1024], DT, tag="psO")
                for g in range(4):
                    pss = psS.tile([P, 1024], DT, tag="psS")
                    for j in range(2):
                        base = HB * j
                        # kx=0: all 256 q
                        nc.tensor.matmul(
                            pss[:, 512 * j : 512 * j + 256],
                            kt[
                                base : base + KA,
                                512 * g + 256 * bb : 512 * g + 256 * bb + 128,
                            ],
                            qt[
                                base : base + KA,
                                512 * g + 256 * bb : 512 * g + 256 * bb + 256,
                            ],
                            start=True,
                            stop=True,
                        )
                        # kx=1: only q in [128, 256) survives the causal mask
                        nc.tensor.matmul(
                            pss[:, 512 * j + 256 : 512 * j + 384],
                            kt[
                                base : base + KA,
                                512 * g + 256 * bb + 128 : 512 * g + 256 * bb + 256,
                            ],
                            qt[
                                base : base + KA,
                                512 * g + 256 * bb + 128 : 512 * g + 256 * bb + 256,
                            ],
                            start=True,
                            stop=True,
                        )
                    pss_r = pss[:].rearrange("p (j r) -> p j r", r=512)
                    pt_r = pt[:, 1024 * g : 1024 * g + 1024].rearrange(
                        "p (j r) -> p j r", r=512
                    )
                    nc.scalar.activation(
                        pt_r[:, :, 0:384], pss_r[:, :, 0:384], AF.Exp
                    )
                    # causal mask: the same 128x128 lower triangle masks all
                    # four regions (j x kx-block), one fused DVE op.  All on
                    # the DVE: keeping GpSimd's queue broadcast-only avoids
                    # FIFO priority inversions against the softmax tail.
                    pt_m = pt[:, 1024 * g : 1024 * g + 1024].rearrange(
                        "p (j s v) -> p j s v", j=2, s=2, v=256
                    )
                    nc.vector.tensor_mul(
                        pt_m[:, :, :, 0:128], pt_m[:, :, :, 0:128], tri_m
                    )

                # the previous batch's projection matmuls slot in here: they
                # are ready to run while this batch's PV waits for the
                # previous softmax-normalize tail to free pso.
                flush_proj()

                for g in range(4):
                    for j in range(2):
                        h = 2 * g + j
                        nc.tensor.matmul(
                            pso[HB * j : HB * j + HB, 256 * g : 256 * g + 256],
                            v_sb[:, HB * h : HB * h + HB],
                            pt[:, 1024 * g + 512 * j : 1024 * g + 512 * j + 256],
                            start=True,
                            stop=False,
                            tile_position=(0, HB * j),
                        )
                        nc.tensor.matmul(
                            pso[
                                HB * j : HB * j + HB,
                                256 * g + 128 : 256 * g + 256,
                            ],
                            v_sb[:, 512 + HB * h : 512 + HB * h + HB],
                            pt[
                                :,
                                1024 * g + 512 * j + 256 : 1024 * g + 512 * j + 384,
                            ],
                            start=False,
                            stop=True,
                            tile_position=(0, HB * j),
                        )

                # softmax denominators live in pso rows 0 (even head of each
                # pair) and 64 (odd head).  1/den = exp(-ln(den)) on ACT --
                # rows 1..63 compute garbage (ln of negatives) that nothing
                # reads.  Both ln and exp live in the natural_log_exp_and_
                # others table set, as do Copy/Identity, so one table load
                # serves the whole kernel.
                # the whole tail is emitted at high priority so the Tile
                # scheduler never slots later-batch work ahead of it on the
                # ACT/DVE/GpSimd queues (observed 15us FIFO inversions).
                with tc.high_priority():
                    lnt = aop.tile([65, 1024], DT, tag="lnt")
                    nc.scalar.activation(lnt[:], pso[0:65, :], AF.Ln)
                    rec = aop.tile([65, 1024], BF, tag="rec")
                    nc.scalar.activation(rec[:], lnt[:], AF.Exp, scale=-1.0)
                    # partition_broadcast ucode: source and dest must start at
                    # partition 0 -- shift the odd-head row 64 -> 0 on the DVE
                    # (cheap, and the DVE demonstrably supports rebasing).
                    rec_o = aop.tile([1, 1024], BF, tag="reco")
                    nc.vector.tensor_copy(rec_o[0:1, :], rec[64:65, :])
                    denb = aop.tile([P, 1024], BF, tag="denb")
                    nc.gpsimd.partition_broadcast(denb[:, :], rec_o[0:1, :])
                    nc.gpsimd.partition_broadcast(denb[0:64, :], rec[0:1, :])
                    ao = aop.tile([P, 1024], MMDT, tag="ao")
                    nc.vector.tensor_mul(ao[:], pso[:], denb[:])

                pending_proj.append((ao, b))

        flush_proj()

        if timing:
            nc.sync.dma_start(ydum_d[:], bp_sb[:, 0:4])

    nc.compile()
    return nc


def make_consts(attn_w, attn_b, proj_w, proj_b):
    attn_w = np.asarray(attn_w, dtype=np.float32)
    attn_b = np.asarray(attn_b, dtype=np.float32)
    proj_w = np.asarray(proj_w, dtype=np.float32)
    proj_b = np.asarray(proj_b, dtype=np.float32)

    s = 1.0 / np.sqrt(HD)
    Wq, Wk, Wv = attn_w[0:C], attn_w[C : 2 * C], attn_w[2 * C : 3 * C]
    bq, bk, bv = attn_b[0:C], attn_b[C : 2 * C], attn_b[2 * C : 3 * C]

    # WQK: [C, 1024] -> [3, 128, 1024]
    M = np.zeros((C, 1024), dtype=np.float32)
    for h in range(NH):
        Wq_h = Wq[HD * h : HD * h + HD]  # [48, C]
        Wk_h = Wk[HD * h : HD * h + HD]
        bq_h = bq[HD * h : HD * h + HD]
        bk_h = bk[HD * h : HD * h + HD]
        # q-hat block
        M[:, HB * h : HB * h + HD] = (s * Wq_h).T
        M[:, HB * h + 48] = s * (bk_h @ Wq_h)  # c_q row
        # (row 49 of q-hat is the ones row via bias)
        # k-hat block
        M[:, 512 + HB * h : 512 + HB * h + HD] = Wk_h.T
        # (row 48 of k-hat is the ones row via bias)
        M[:, 512 + HB * h + 49] = s * (bq_h @ Wk_h)  # c_k row
    WQK = np.ascontiguousarray(M.reshape(C, 1024).reshape(3, P, 1024))

    # WV: [C, 384] tightly packed -- col 48*h+j = Wv row HD*h+j (= Wv.T)
    WV = np.ascontiguousarray(Wv.T.reshape(C, 384).reshape(3, P, 384))

    # WP: [512, 384] -> [4, 128, 384]; row HB*h + 1 + j = proj_w[:, HD*h+j].
    # Row 0 of head-block 0 carries the effective projection bias: after
    # normalization the denominator row of ao is exactly 1.0.
    bp_eff = proj_b + proj_w @ bv
    Wp_aug = np.zeros((512, C), dtype=np.float32)
    Wp_aug[0, :] = bp_eff
    for h in range(NH):
        Wp_aug[HB * h + 1 : HB * h + 1 + HD, :] = proj_w[:, HD * h : HD * h + HD].T
    WP = np.ascontiguousarray(Wp_aug.reshape(4, P, 384))

    BQ = np.zeros((P, 1), dtype=np.float32)
    BQ[49, 0] = 1.0
    BQ[49 + HB, 0] = 1.0
    BK = np.zeros((P, 1), dtype=np.float32)
    BK[48, 0] = 1.0
    BK[48 + HB, 0] = 1.0

    # causal 0/1 mask for S^T[k, q] tiles (x4 for the j x kx-block regions)
    kk = np.arange(128)[:, None]
    qq = np.arange(128)[None, :]
    tri = (qq >= kk).astype(np.float32)  # [128k, 128q]
    TRI0 = np.ascontiguousarray(np.concatenate([tri] * 4, axis=1)).astype(BF16_NP)

    BP = np.ascontiguousarray(np.broadcast_to(bp_eff[None, :], (P, 384))).astype(
        np.float32
    )

    # v-init pattern: ones column at 64h, EPS_PAD at cols 49..63 of each block
    vinit_row = np.zeros(1024, dtype=np.float32)
    for kx in range(2):
        for h in range(NH):
            off = 512 * kx + HB * h
            vinit_row[off] = 1.0
            vinit_row[off + 49 : off + HB] = EPS_PAD
    VINIT = np.ascontiguousarray(np.broadcast_to(vinit_row[None, :], (P, 1024))).astype(
        BF16_NP
    )

    WQK = WQK.astype(BF16_NP)
    WV = WV.astype(BF16_NP)
    WP = WP.astype(BF16_NP)

    return {
        "vinit": VINIT,
        "wqk": WQK,
        "wv": WV,
        "wp": WP,
        "bq": BQ,
        "bk": BK,
        "tri0": TRI0,
        "bp": BP,
    }


_NC_CACHE = {}


def get_nc(nb: int = NB):
    if nb not in _NC_CACHE:
        _NC_CACHE[nb] = build_nc(nb)
    return _NC_CACHE[nb]


def make_in_maps(x, attn_w, attn_b, proj_w, proj_b):
    x = np.asarray(x, dtype=np.float32)
    consts = make_consts(attn_w, attn_b, proj_w, proj_b)
    in_maps = []
    for core in range(N_CORES):
        xs = x[core * NB : (core + 1) * NB]  # [NB, T, C]
        # [3, 128, NB, T]: xTl[ci, p, b, t] = x[b, t, 128*ci + p]
        xTl = np.ascontiguousarray(
            xs.transpose(2, 0, 1).reshape(3, P, NB, T)
        ).astype(BF16_NP)
        m = {"xT": xTl}
        m.update(consts)
        in_maps.append(m)
    return in_maps


def kernel(x, attn_w, attn_b, proj_w, proj_b):
    nc = get_nc(NB)
    in_maps = make_in_maps(x, attn_w, attn_b, proj_w, proj_b)
    res = run_bass_kernel_spmd(nc, in_maps, core_ids=list(range(N_CORES)))
    out = np.concatenate(
        [res.results[i]["y"] for i in range(N_CORES)], axis=0
    ).astype(np.float32)
    return out
